# revision 4
# baseline (speedup 1.0000x reference)
"""Trainium2 Bass kernel for nn_EnhanceDiversityFeatureExtracition.

Computes  loss = mean((x-y)^2) + ALPHA * diversity_reg(conv_w)
where diversity_reg builds a 64x64 Gram matrix of the F=64 slices
conv_w[:, :, i, :] (each flattened to a 786432-vector), normalizes it to
cosine similarities, and sums the entries with tau < sim <= 1 off the
diagonal.

Distribution (8 NeuronCores, SPMD):
  - x_batch / y_batch sharded on batch dim: 256 rows per core.
  - conv_w viewed as A = conv_w.reshape(262144, 192)  (row m = (o,c),
    col = f*3+k).  gram[i,j] = sum_m sum_k A[m,3i+k]*A[m,3j+k], so A is
    sharded along the 262144-row reduction axis: 32768 rows per core.
  - Each core returns a partial 64x64 gram and per-partition partial
    sums of (x-y)^2; the host sums the partials and applies the tiny
    64x64 masked-similarity epilogue.

On-core dataflow:
  - A shard is streamed in 32 blocks of 1024 rows laid out as
    [128 partitions x 1536 floats] (per-partition contiguous 6KB DMA).
    Each block yields 8*3 = 24 matmuls (contraction 128, strided
    64-wide k-slices as both operands) accumulating into one 64x64
    PSUM tile across the whole shard.
  - MSE: 4 chunks of [128 x 2048] per operand; DVE computes d = x-y,
    ACT computes Square(d) with a per-partition accumulate.
"""

import numpy as np

import concourse.bass as bass
import concourse.mybir as mybir
from concourse import bacc, tile
from concourse.bass_utils import run_bass_kernel_spmd

N_CORES = 8
B, D = 2048, 4096            # x_batch / y_batch
M, G = 262144, 192           # conv_w as (M, G); G = F*KW
F, KW = 64, 3
ROWS = B // N_CORES          # 256 batch rows per core
MC = M // N_CORES            # 32768 reduction rows per core
TPB = 8                      # 128-row tiles per DMA block
BLK = 128 * TPB              # 1024 rows per block
NBLK = MC // BLK             # 32
NCH = 4                      # MSE chunks per core
CHW = (ROWS * D) // (128 * NCH)  # 2048 floats per partition per chunk

ALPHA = 0.0005
TAU = 0.2

_prog = None


def _build() -> bass.Bass:
    nc = bacc.Bacc(None, target_bir_lowering=False)
    f32 = mybir.dt.float32

    xs = nc.dram_tensor("xs", [ROWS, D], f32, kind="ExternalInput")
    ys = nc.dram_tensor("ys", [ROWS, D], f32, kind="ExternalInput")
    aw = nc.dram_tensor("aw", [MC, G], f32, kind="ExternalInput")
    gram_part = nc.dram_tensor("gram_part", [F, F], f32, kind="ExternalOutput")
    sse_part = nc.dram_tensor("sse_part", [128, NCH], f32, kind="ExternalOutput")

    with tile.TileContext(nc) as tc:
        with (
            tc.tile_pool(name="apool", bufs=4) as apool,
            tc.tile_pool(name="xpool", bufs=2) as xpool,
            tc.tile_pool(name="ypool", bufs=2) as ypool,
            tc.tile_pool(name="dpool", bufs=2) as dpool,
            tc.tile_pool(name="qpool", bufs=2) as qpool,
            tc.tile_pool(name="opool", bufs=1) as opool,
            tc.tile_pool(name="psum", bufs=1, space=bass.MemorySpace.PSUM) as psum,
        ):
            gps = psum.tile([F, F], f32)
            acc = opool.tile([128, NCH], f32)

            # per-partition contiguous views
            awv = aw[:].rearrange("(b p t) g -> b p (t g)", p=128, t=TPB)
            xv = xs[:].rearrange("(p t) d -> p (t d)", p=128)
            yv = ys[:].rearrange("(p t) d -> p (t d)", p=128)

            n_mm = NBLK * TPB * KW
            mm = 0
            for b in range(NBLK):
                at = apool.tile([128, TPB * G], f32)
                nc.sync.dma_start(at[:], awv[b])
                at3 = at[:].rearrange("p (t g) -> p t g", t=TPB)
                for t in range(TPB):
                    for k in range(KW):
                        sl = at3[:, t, k::KW]
                        nc.tensor.matmul(
                            gps[:], sl, sl,
                            start=(mm == 0), stop=(mm == n_mm - 1),
                        )
                        mm += 1

                # interleave the MSE chunks among the first A blocks
                if b < NCH:
                    ch = b
                    xt = xpool.tile([128, CHW], f32)
                    nc.sync.dma_start(xt[:], xv[:, ch * CHW:(ch + 1) * CHW])
                    yt = ypool.tile([128, CHW], f32)
                    nc.sync.dma_start(yt[:], yv[:, ch * CHW:(ch + 1) * CHW])
                    dtile = dpool.tile([128, CHW], f32)
                    nc.vector.tensor_sub(dtile[:], xt[:], yt[:])
                    qtile = qpool.tile([128, CHW], f32)
                    nc.scalar.activation(
                        qtile[:], dtile[:],
                        mybir.ActivationFunctionType.Square,
                        accum_out=acc[:, ch:ch + 1],
                    )

            gsb = opool.tile([F, F], f32)
            nc.vector.tensor_copy(gsb[:], gps[:])
            nc.sync.dma_start(gram_part[:], gsb[:])
            nc.sync.dma_start(sse_part[:], acc[:])

    nc.finalize()
    return nc


def _get_prog() -> bass.Bass:
    global _prog
    if _prog is None:
        _prog = _build()
    return _prog


def _epilogue(gram: np.ndarray, sse: float) -> np.ndarray:
    norms = np.sqrt(np.diag(gram))
    sim = gram / np.outer(norms, norms)
    mask = (sim > TAU) & (sim <= 1.0) & (~np.eye(F, dtype=bool))
    reg = sim[mask].sum()
    loss = sse / float(B * D) + ALPHA * reg
    return np.asarray(np.float32(loss))


def kernel(x_batch: np.ndarray, y_batch: np.ndarray, conv_w: np.ndarray) -> np.ndarray:
    nc = _get_prog()
    A = np.ascontiguousarray(conv_w.reshape(M, G))
    in_maps = []
    for c in range(N_CORES):
        in_maps.append({
            "xs": np.ascontiguousarray(x_batch[c * ROWS:(c + 1) * ROWS]),
            "ys": np.ascontiguousarray(y_batch[c * ROWS:(c + 1) * ROWS]),
            "aw": np.ascontiguousarray(A[c * MC:(c + 1) * MC]),
        })
    res = run_bass_kernel_spmd(nc, in_maps, core_ids=list(range(N_CORES))).results
    gram = np.zeros((F, F), np.float64)
    sse = 0.0
    for r in res:
        gram += r["gram_part"].astype(np.float64)
        sse += float(r["sse_part"].sum(dtype=np.float64))
    return _epilogue(gram, sse)


# revision 5
# speedup vs baseline: 1.4099x; 1.4099x over previous
"""Trainium2 Bass kernel for nn_EnhanceDiversityFeatureExtracition.

Computes  loss = mean((x-y)^2) + ALPHA * diversity_reg(conv_w)
where diversity_reg builds a 64x64 Gram matrix of the F=64 slices
conv_w[:, :, i, :] (each flattened to a 786432-vector), normalizes it to
cosine similarities, and sums the entries with tau < sim <= 1 off the
diagonal.

Distribution (8 NeuronCores, SPMD):
  - x_batch / y_batch sharded on batch dim: 256 rows per core.
  - conv_w viewed as A = conv_w.reshape(262144, 192)  (row m = (o,c),
    col = f*3+k).  gram[i,j] = sum_m sum_k A[m,3i+k]*A[m,3j+k], so A is
    sharded along the 262144-row reduction axis: 32768 rows per core.
  - Each core returns a partial 64x64 gram and per-partition partial
    sums of (x-y)^2; the host sums the partials and applies the tiny
    64x64 masked-similarity epilogue.

On-core dataflow:
  - A shard is streamed in 32 blocks of 1024 rows laid out as
    [128 partitions x 1536 floats] (per-partition contiguous 6KB DMA).
    Each block yields 8*3 = 24 matmuls (contraction 128, strided
    64-wide k-slices as both operands) accumulating into one 64x64
    PSUM tile across the whole shard.
  - MSE: 4 chunks of [128 x 2048] per operand; DVE computes d = x-y,
    ACT computes Square(d) with a per-partition accumulate.
"""

import numpy as np

import concourse.bass as bass
import concourse.mybir as mybir
from concourse import bacc, tile
from concourse.bass_utils import run_bass_kernel_spmd

N_CORES = 8
B, D = 2048, 4096            # x_batch / y_batch
M, G = 262144, 192           # conv_w as (M, G); G = F*KW
F, KW = 64, 3
ROWS = B // N_CORES          # 256 batch rows per core
MC = M // N_CORES            # 32768 reduction rows per core
TPB = 8                      # 128-row tiles per DMA block
BLK = 128 * TPB              # 1024 rows per block
NBLK = MC // BLK             # 32
NCH = 4                      # MSE chunks per core
CHW = (ROWS * D) // (128 * NCH)  # 2048 floats per partition per chunk

ALPHA = 0.0005
TAU = 0.2

_prog = None


def _build() -> bass.Bass:
    nc = bacc.Bacc(None, target_bir_lowering=False)
    f32 = mybir.dt.float32

    xs = nc.dram_tensor("xs", [ROWS, D], f32, kind="ExternalInput")
    ys = nc.dram_tensor("ys", [ROWS, D], f32, kind="ExternalInput")
    aw = nc.dram_tensor("aw", [MC, G], f32, kind="ExternalInput")
    gram_part = nc.dram_tensor("gram_part", [F, F], f32, kind="ExternalOutput")
    sse_part = nc.dram_tensor("sse_part", [128, NCH], f32, kind="ExternalOutput")

    bf16 = mybir.dt.bfloat16
    with tile.TileContext(nc) as tc:
        with (
            tc.tile_pool(name="apool", bufs=6) as apool,
            tc.tile_pool(name="bpool", bufs=4) as bpool,
            tc.tile_pool(name="xpool", bufs=2) as xpool,
            tc.tile_pool(name="ypool", bufs=2) as ypool,
            tc.tile_pool(name="dpool", bufs=2) as dpool,
            tc.tile_pool(name="qpool", bufs=2) as qpool,
            tc.tile_pool(name="opool", bufs=1) as opool,
            tc.tile_pool(name="psum", bufs=1, space=bass.MemorySpace.PSUM) as psum,
        ):
            gps = psum.tile([F, F], f32)
            acc = opool.tile([128, NCH], f32)

            # per-partition contiguous views
            awv = aw[:].rearrange("(b p t) g -> b p (t g)", p=128, t=TPB)
            xv = xs[:].rearrange("(p t) d -> p (t d)", p=128)
            yv = ys[:].rearrange("(p t) d -> p (t d)", p=128)

            # round-robin the fp32->bf16 cast across otherwise-idle engines
            cast_engines = [nc.vector, nc.gpsimd, nc.vector, nc.gpsimd, nc.scalar]

            n_mm = NBLK * TPB * KW
            mm = 0
            for b in range(NBLK):
                at = apool.tile([128, TPB * G], f32)
                nc.sync.dma_start(at[:], awv[b])
                bt = bpool.tile([128, TPB * G], bf16)
                eng = cast_engines[b % len(cast_engines)]
                if eng is nc.scalar:
                    nc.scalar.copy(bt[:], at[:])
                else:
                    eng.tensor_copy(bt[:], at[:])
                bt3 = bt[:].rearrange("p (t g) -> p t g", t=TPB)
                for t in range(TPB):
                    for k in range(KW):
                        sl = bt3[:, t, k::KW]
                        nc.tensor.matmul(
                            gps[:], sl, sl,
                            start=(mm == 0), stop=(mm == n_mm - 1),
                        )
                        mm += 1

                # interleave the MSE chunks among the first A blocks
                if b < NCH:
                    ch = b
                    xt = xpool.tile([128, CHW], f32)
                    nc.sync.dma_start(xt[:], xv[:, ch * CHW:(ch + 1) * CHW])
                    yt = ypool.tile([128, CHW], f32)
                    nc.sync.dma_start(yt[:], yv[:, ch * CHW:(ch + 1) * CHW])
                    dtile = dpool.tile([128, CHW], f32)
                    nc.vector.tensor_sub(dtile[:], xt[:], yt[:])
                    qtile = qpool.tile([128, CHW], f32)
                    nc.scalar.activation(
                        qtile[:], dtile[:],
                        mybir.ActivationFunctionType.Square,
                        accum_out=acc[:, ch:ch + 1],
                    )

            gsb = opool.tile([F, F], f32)
            nc.vector.tensor_copy(gsb[:], gps[:])
            nc.sync.dma_start(gram_part[:], gsb[:])
            nc.sync.dma_start(sse_part[:], acc[:])

    nc.finalize()
    return nc


def _get_prog() -> bass.Bass:
    global _prog
    if _prog is None:
        _prog = _build()
    return _prog


def _epilogue(gram: np.ndarray, sse: float) -> np.ndarray:
    norms = np.sqrt(np.diag(gram))
    sim = gram / np.outer(norms, norms)
    mask = (sim > TAU) & (sim <= 1.0) & (~np.eye(F, dtype=bool))
    reg = sim[mask].sum()
    loss = sse / float(B * D) + ALPHA * reg
    return np.asarray(np.float32(loss))


def kernel(x_batch: np.ndarray, y_batch: np.ndarray, conv_w: np.ndarray) -> np.ndarray:
    nc = _get_prog()
    A = np.ascontiguousarray(conv_w.reshape(M, G))
    in_maps = []
    for c in range(N_CORES):
        in_maps.append({
            "xs": np.ascontiguousarray(x_batch[c * ROWS:(c + 1) * ROWS]),
            "ys": np.ascontiguousarray(y_batch[c * ROWS:(c + 1) * ROWS]),
            "aw": np.ascontiguousarray(A[c * MC:(c + 1) * MC]),
        })
    res = run_bass_kernel_spmd(nc, in_maps, core_ids=list(range(N_CORES))).results
    gram = np.zeros((F, F), np.float64)
    sse = 0.0
    for r in res:
        gram += r["gram_part"].astype(np.float64)
        sse += float(r["sse_part"].sum(dtype=np.float64))
    return _epilogue(gram, sse)


# revision 9
# speedup vs baseline: 1.6550x; 1.1739x over previous
"""Trainium2 Bass kernel for nn_EnhanceDiversityFeatureExtracition.

Computes  loss = mean((x-y)^2) + ALPHA * diversity_reg(conv_w)
where diversity_reg builds a 64x64 Gram matrix of the F=64 slices
conv_w[:, :, i, :] (each flattened to a 786432-vector), normalizes it to
cosine similarities, and sums the entries with tau < sim <= 1 off the
diagonal.

Distribution (8 NeuronCores, SPMD):
  - x_batch / y_batch sharded on batch dim: 256 rows per core.
  - conv_w viewed as A = conv_w.reshape(262144, 192)  (row m = (o,c),
    col = f*3+k).  gram[i,j] = sum_m sum_k A[m,3i+k]*A[m,3j+k], so A is
    sharded along the 262144-row reduction axis: 32768 rows per core.
  - Each core returns a partial 64x64 gram and per-partition partial
    sums of (x-y)^2; the host sums the partials and applies the tiny
    64x64 masked-similarity epilogue.

On-core dataflow:
  - A shard is streamed in 32 blocks of 1024 rows laid out as
    [128 partitions x 1536 floats] (per-partition contiguous 6KB DMA).
    Each block yields 8*3 = 24 matmuls (contraction 128, strided
    64-wide k-slices as both operands) accumulating into one 64x64
    PSUM tile across the whole shard.
  - MSE: 4 chunks of [128 x 2048] per operand; DVE computes d = x-y,
    ACT computes Square(d) with a per-partition accumulate.
"""

import numpy as np

import concourse.bass as bass
import concourse.mybir as mybir
from concourse import bacc, tile
from concourse.bass_utils import run_bass_kernel_spmd

N_CORES = 8
B, D = 2048, 4096            # x_batch / y_batch
M, G = 262144, 192           # conv_w as (M, G); G = F*KW
F, KW = 64, 3
ROWS = B // N_CORES          # 256 batch rows per core
MC = M // N_CORES            # 32768 reduction rows per core
TPB = 8                      # 128-row tiles per DMA block
BLK = 128 * TPB              # 1024 rows per block
NBLK = MC // BLK             # 32
NCH = 4                      # MSE chunks per core
CHW = (ROWS * D) // (128 * NCH)  # 2048 floats per partition per chunk

ALPHA = 0.0005
TAU = 0.2

_prog = None


def _build() -> bass.Bass:
    nc = bacc.Bacc(None, target_bir_lowering=False)
    f32 = mybir.dt.float32

    xs = nc.dram_tensor("xs", [ROWS, D], f32, kind="ExternalInput")
    ys = nc.dram_tensor("ys", [ROWS, D], f32, kind="ExternalInput")
    aw = nc.dram_tensor("aw", [MC, G], mybir.dt.float32r, kind="ExternalInput")
    gram_part = nc.dram_tensor("gram_part", [F, F], f32, kind="ExternalOutput")
    sse_part = nc.dram_tensor("sse_part", [128, NCH], f32, kind="ExternalOutput")

    f32r = mybir.dt.float32r

    # Selection matrix for the k-diagonal extraction:
    # S[3j+k, 64k+j] = 1, so (S^T C S)-style products give
    # gram[i,j] = sum_k C[3i+k, 3j+k].
    S = np.zeros((G, G), np.float32)
    for k in range(KW):
        for j in range(F):
            S[KW * j + k, F * k + j] = 1.0
    s_dram = nc.inline_tensor(S, name="sel_const")

    # moving operand width for the fp32r full-rate mode
    RW = 256
    PAD = RW - G  # 64 junk columns beyond each 192-col tile

    with tile.TileContext(nc) as tc:
        with (
            tc.tile_pool(name="apool", bufs=6) as apool,
            tc.tile_pool(name="xpool", bufs=2) as xpool,
            tc.tile_pool(name="ypool", bufs=2) as ypool,
            tc.tile_pool(name="dpool", bufs=2) as dpool,
            tc.tile_pool(name="qpool", bufs=2) as qpool,
            tc.tile_pool(name="opool", bufs=1) as opool,
            tc.tile_pool(name="spool", bufs=1) as spool,
            tc.tile_pool(name="psum", bufs=1, space=bass.MemorySpace.PSUM) as psum,
        ):
            # C = A^T A accumulator, rows 0-127 and 128-191
            cps1 = psum.tile([128, RW], f32, tag="cps1")
            cps2 = psum.tile([F, RW], f32, tag="cps2")
            acc = opool.tile([128, NCH], f32)

            # per-partition contiguous views
            awv = aw[:].rearrange("(b p t) g -> b p (t g)", p=128, t=TPB)
            xv = xs[:].rearrange("(p t) d -> p (t d)", p=128)
            yv = ys[:].rearrange("(p t) d -> p (t d)", p=128)

            n_t = NBLK * TPB
            ti = 0
            for b in range(NBLK):
                # A block, padded so the last sub-tile's 256-wide moving
                # operand stays in bounds
                at = apool.tile([128, TPB * G + PAD], f32r)
                nc.sync.dma_start(at[:, :TPB * G], awv[b])
                nc.gpsimd.memset(at[:, TPB * G:].bitcast(f32), 0.0)
                for t in range(TPB):
                    rhs = at[:, t * G:t * G + RW]
                    w1 = at[:, t * G:t * G + 128]
                    w2 = at[:, t * G + 128:t * G + G]
                    nc.tensor.matmul(
                        cps1[:], w1, rhs,
                        start=(ti == 0), stop=(ti == n_t - 1),
                    )
                    nc.tensor.matmul(
                        cps2[:], w2, rhs,
                        start=(ti == 0), stop=(ti == n_t - 1),
                    )
                    ti += 1

                # interleave the MSE chunks among the first A blocks
                if b < NCH:
                    ch = b
                    xt = xpool.tile([128, CHW], f32)
                    nc.sync.dma_start(xt[:], xv[:, ch * CHW:(ch + 1) * CHW])
                    yt = ypool.tile([128, CHW], f32)
                    nc.sync.dma_start(yt[:], yv[:, ch * CHW:(ch + 1) * CHW])
                    dtile = dpool.tile([128, CHW], f32)
                    nc.vector.tensor_sub(dtile[:], xt[:], yt[:])
                    qtile = qpool.tile([128, CHW], f32)
                    nc.scalar.activation(
                        qtile[:], dtile[:],
                        mybir.ActivationFunctionType.Square,
                        accum_out=acc[:, ch:ch + 1],
                    )

            # ---- extract gram[i,j] = sum_k C[3i+k, 3j+k] via selection matmuls
            ssb1r = spool.tile([128, G], f32, tag="ssb1r")
            nc.sync.dma_start(ssb1r[:], s_dram[0:128, :])
            ssb1 = spool.tile([128, G], f32r, tag="ssb1")
            nc.vector.tensor_copy(ssb1[:], ssb1r[:])
            ssb2r = spool.tile([F, G], f32, tag="ssb2r")
            nc.sync.dma_start(ssb2r[:], s_dram[128:G, :])
            ssb2 = spool.tile([F, G], f32r, tag="ssb2")
            nc.vector.tensor_copy(ssb2[:], ssb2r[:])
            csb1 = opool.tile([128, G], f32r, tag="csb1")
            nc.vector.tensor_copy(csb1[:], cps1[:, :G])
            csb2 = opool.tile([F, G], f32r, tag="csb2")
            nc.vector.tensor_copy(csb2[:], cps2[:, :G])

            gps = psum.tile([F, F], f32, tag="gps")
            n_sel = 2 * KW
            si = 0
            for k in range(KW):
                for ssb, csb in ((ssb1, csb1), (ssb2, csb2)):
                    nc.tensor.matmul(
                        gps[:],
                        ssb[:, F * k:F * (k + 1)],
                        csb[:, k::KW],
                        start=(si == 0), stop=(si == n_sel - 1),
                    )
                    si += 1

            gsb = opool.tile([F, F], f32, tag="gsb")
            nc.vector.tensor_copy(gsb[:], gps[:])
            nc.sync.dma_start(gram_part[:], gsb[:])
            nc.sync.dma_start(sse_part[:], acc[:])

    nc.finalize()
    return nc


def _get_prog() -> bass.Bass:
    global _prog
    if _prog is None:
        _prog = _build()
    return _prog


def _epilogue(gram: np.ndarray, sse: float) -> np.ndarray:
    norms = np.sqrt(np.diag(gram))
    sim = gram / np.outer(norms, norms)
    mask = (sim > TAU) & (sim <= 1.0) & (~np.eye(F, dtype=bool))
    reg = sim[mask].sum()
    loss = sse / float(B * D) + ALPHA * reg
    return np.asarray(np.float32(loss))


def kernel(x_batch: np.ndarray, y_batch: np.ndarray, conv_w: np.ndarray) -> np.ndarray:
    nc = _get_prog()
    A = np.ascontiguousarray(conv_w.reshape(M, G))
    in_maps = []
    for c in range(N_CORES):
        in_maps.append({
            "xs": np.ascontiguousarray(x_batch[c * ROWS:(c + 1) * ROWS]),
            "ys": np.ascontiguousarray(y_batch[c * ROWS:(c + 1) * ROWS]),
            "aw": np.ascontiguousarray(A[c * MC:(c + 1) * MC]),
        })
    res = run_bass_kernel_spmd(nc, in_maps, core_ids=list(range(N_CORES))).results
    gram = np.zeros((F, F), np.float64)
    sse = 0.0
    for r in res:
        gram += r["gram_part"].astype(np.float64)
        sse += float(r["sse_part"].sum(dtype=np.float64))
    return _epilogue(gram, sse)
